# revision 10
# baseline (speedup 1.0000x reference)
"""ContactMapHead bilinear pair-scoring kernel for 8 trn2 NeuronCores.

Math: for each batch b, logits[b, p] = h[b, i_p] @ W @ h[b, j_p] + bias,
where (i_p, j_p) enumerate position pairs (upper triangle, k=1, when the
masks keep every position — the general case is handled too).

This equals S_b = (h_b @ W) @ h_b^T + bias followed by a pair gather.
Sharding (8 cores): core c computes rows [r0, r0+128) of S_b for batch
b = c // 4, r0 = (c % 4) * 128.  W and h_b^T are replicated per core; the
host assembles S (2, 512, 512) from the 8 row-blocks and gathers the
pair indices (pure unshard/reindex).

Perf notes (v4, from ntff traces):
  - All inputs bf16: the kernel is DMA-bound and bf16 halves the bytes;
    bf16 matmuls are also single-pass on the PE (fp32 is double-pumped).
  - Per-queue DMA throughput is the binding constraint, not aggregate
    HBM: each HWDGE queue (sync/scalar rings) sustains only ~55-85 GB/s,
    while the gpsimd software-DGE queue sustains ~230 GB/s after a ~2us
    cold start.  So the critical chain (bias warms the SWDGE up, then
    hsrt -> w front -> hst front) rides the gpsimd queue, and the two
    back halves ride the slower HW queues in parallel.
  - Fine-grained semaphores let stage 1 overlap the remaining input DMA,
    and stage 2 is interleaved with the tail of stage 1 on the PE.
  - Stage 2 accumulates into two psum banks (column halves) so the two
    bias-adds (DVE + Act engines, parallel) and the two out DMAs start
    before the last matmul retires.
  - PE warmup matmuls on zeros keep the HAM clock gate open while the
    input DMAs are in flight.

Device tensors (per core), all swizzled host-side so every DMA row is a
contiguous run (>=1KB per partition):
    w    (128, 2048) bf16: w[p, kc*512 + c]   = W[kc*128 + p, c]
    hst  (128, 2048) bf16: hst[p, hc*512 + j] = h_b[j, hc*128 + p]
    hsrt (128, 512)  bf16: hsrt[p, kc*128 + m] = h_b[r0 + m, kc*128 + p]
    bias (1,) f32
    out  (128, 512) f32:  out[m, j] = S_b[r0 + m, j] + bias
"""

import numpy as np

_B, _L, _H = 2, 512, 512
_P = 128
_KC = _H // _P          # 4 contraction chunks
_GROUPS = 4             # row-blocks per batch
_RB = _L // _GROUPS     # 128 rows per core
_NCORES = 8
_NWARM = 20             # warmup matmuls (~107ns pitch each at full clock)

# Dev/profiling knobs (used by test.py only; harness leaves them alone).
TRACE = False
TRACE_KWARGS = {}
LAST_RESULTS = None

_STATE = {}


def _build_nc():
    """Build (once) the raw-bass module for one core's row-block."""
    if "nc" in _STATE:
        return _STATE["nc"]

    from concourse import bacc, mybir

    f32 = mybir.dt.float32
    bf16 = mybir.dt.bfloat16
    nc = bacc.Bacc("TRN2", target_bir_lowering=False, debug=False)

    w_d = nc.dram_tensor("w", [_P, 2048], bf16, kind="ExternalInput")
    hst_d = nc.dram_tensor("hst", [_P, 2048], bf16, kind="ExternalInput")
    hsrt_d = nc.dram_tensor("hsrt", [_P, 512], bf16, kind="ExternalInput")
    bias_d = nc.dram_tensor("bias", [1], f32, kind="ExternalInput")
    out_d = nc.dram_tensor("out", [_RB, _L], f32, kind="ExternalOutput")

    w_sb = nc.alloc_sbuf_tensor("w_sb", [_P, 2048], bf16)
    hst_sb = nc.alloc_sbuf_tensor("hst_sb", [_P, 2048], bf16)
    hsrt_sb = nc.alloc_sbuf_tensor("hsrt_sb", [_P, 512], bf16)
    bias_sb = nc.alloc_sbuf_tensor("bias_sb", [_P, 1], f32)
    gt_sb = nc.alloc_sbuf_tensor("gt_sb", [_P, 512], bf16)
    out_sb = nc.alloc_sbuf_tensor("out_sb", [_P, _L], f32)
    warm_sb = nc.alloc_sbuf_tensor("warm_sb", [_P, _P], bf16)
    pgt = [nc.alloc_psum_tensor(f"pgt{h}", [_P, _P], f32) for h in range(_KC)]
    ps0 = nc.alloc_psum_tensor("ps0", [_P, 256], f32)
    ps1 = nc.alloc_psum_tensor("ps1", [_P, 256], f32)
    pwarm = nc.alloc_psum_tensor("pwarm", [_P, _P], f32)

    s_wf = nc.alloc_semaphore("s_wf")      # +16 w front (kc 0,1)
    s_wb = nc.alloc_semaphore("s_wb")      # +16 w back (kc 2,3)
    s_hf = nc.alloc_semaphore("s_hf")      # +16 hst front (hc 0,1)
    s_hb = nc.alloc_semaphore("s_hb")      # +16 hst back (hc 2,3)
    s_hr = nc.alloc_semaphore("s_hr")      # +16 hsrt
    s_bias = nc.alloc_semaphore("s_bias")  # +16 bias
    s_gt_pe = nc.alloc_semaphore("s_gt_pe")  # +1 per stage-1 group done
    s_gt_v = nc.alloc_semaphore("s_gt_v")    # +1 per gt copy
    s_s2a = nc.alloc_semaphore("s_s2a")    # +1 ps0 (cols 0:256) done
    s_s2b = nc.alloc_semaphore("s_s2b")    # +1 ps1 (cols 256:512) done
    s_out0 = nc.alloc_semaphore("s_out0")  # +1 bias-add half 0
    s_out1 = nc.alloc_semaphore("s_out1")  # +1 bias-add half 1
    s_od = nc.alloc_semaphore("s_od")      # +16 per out-DMA half
    s_warm = nc.alloc_semaphore("s_warm")  # +1 warmup scratch zeroed

    with nc.Block(no_gpsimd_drain=True) as block:

        @block.sync
        def _(sync):
            sync.dma_start(out=w_sb[:, 1024:2048], in_=w_d[:, 1024:2048]).then_inc(
                s_wb, 16
            )
            sync.wait_ge(s_od, 32)

        @block.scalar
        def _(scalar):
            scalar.dma_start(
                out=hst_sb[:, 1024:2048], in_=hst_d[:, 1024:2048]
            ).then_inc(s_hb, 16)
            # bias-add half 1 on the Act engine (parallel to DVE's half 0)
            scalar.wait_ge(s_s2b, 1)
            scalar.wait_ge(s_bias, 16)
            nc.scalar.activation(
                out_sb[:, 256:512],
                ps1[:],
                mybir.ActivationFunctionType.Identity,
                bias=bias_sb[:, 0:1],
            ).then_inc(s_out1, 1)
            scalar.wait_ge(s_od, 32)

        @block.gpsimd
        def _(gpsimd):
            nc.gpsimd.memset(warm_sb[:], 0.0).then_inc(s_warm, 1)
            # bias first: tiny transfer soaks the SWDGE cold-start latency
            gpsimd.dma_start(
                out=bias_sb[:], in_=bias_d[:].to_broadcast((_P, 1))
            ).then_inc(s_bias, 16)
            gpsimd.dma_start(out=hsrt_sb[:], in_=hsrt_d[:]).then_inc(s_hr, 16)
            gpsimd.dma_start(out=w_sb[:, 0:1024], in_=w_d[:, 0:1024]).then_inc(
                s_wf, 16
            )
            gpsimd.dma_start(out=hst_sb[:, 0:1024], in_=hst_d[:, 0:1024]).then_inc(
                s_hf, 16
            )
            # outputs ride the (warm, fast) SWDGE queue too
            gpsimd.wait_ge(s_out0, 1)
            gpsimd.dma_start(out=out_d[:, 0:256], in_=out_sb[:, 0:256]).then_inc(
                s_od, 16
            )
            gpsimd.wait_ge(s_out1, 1)
            gpsimd.dma_start(out=out_d[:, 256:512], in_=out_sb[:, 256:512]).then_inc(
                s_od, 16
            )
            gpsimd.wait_ge(s_od, 32)

        @block.tensor
        def _(tensor):
            # HAM warmup: keep the PE busy on zeros so the clock gate opens
            # (1.2 -> 2.4 GHz) while the input DMAs are in flight.
            tensor.wait_ge(s_warm, 1)
            for _i in range(_NWARM):
                nc.tensor.matmul(
                    pwarm[:], lhsT=warm_sb[:], rhs=warm_sb[:], start=True, stop=True
                )

            def s1(hc, kc):
                return nc.tensor.matmul(
                    pgt[hc][:],
                    lhsT=w_sb[:, kc * 512 + hc * _P : kc * 512 + (hc + 1) * _P],
                    rhs=hsrt_sb[:, kc * _P : (kc + 1) * _P],
                    start=(kc == 0),
                    stop=(kc == _KC - 1),
                )

            def s2(j):
                # two psum banks (column halves) so the bias-adds + out DMAs
                # can start before the second half finishes
                tensor.wait_ge(s_gt_v, j + 1)
                tensor.wait_ge(s_hf if j < 2 else s_hb, 16)
                mm_a = nc.tensor.matmul(
                    ps0[:],
                    lhsT=gt_sb[:, j * _P : (j + 1) * _P],
                    rhs=hst_sb[:, j * 512 : j * 512 + 256],
                    start=(j == 0),
                    stop=(j == _KC - 1),
                )
                mm_b = nc.tensor.matmul(
                    ps1[:],
                    lhsT=gt_sb[:, j * _P : (j + 1) * _P],
                    rhs=hst_sb[:, j * 512 + 256 : (j + 1) * 512],
                    start=(j == 0),
                    stop=(j == _KC - 1),
                )
                if j == _KC - 1:
                    mm_a.then_inc(s_s2a, 1)
                    mm_b.then_inc(s_s2b, 1)

            # stage 1 on the front halves (kc 0,1) — overlaps back DMAs
            tensor.wait_ge(s_wf, 16)
            tensor.wait_ge(s_hr, 16)
            for hc in range(_KC):
                for kc in (0, 1):
                    s1(hc, kc)
            # finish each group on the back half, interleaving stage 2
            tensor.wait_ge(s_wb, 16)
            for hc in range(_KC):
                s1(hc, 2)
                s1(hc, 3).then_inc(s_gt_pe, 1)
                if hc >= 2:
                    s2(hc - 2)
            s2(_KC - 2)
            s2(_KC - 1)

        @block.vector
        def _(vector):
            for hc in range(_KC):
                vector.wait_ge(s_gt_pe, hc + 1)
                nc.vector.tensor_copy(
                    gt_sb[:, hc * _P : (hc + 1) * _P], pgt[hc][:]
                ).then_inc(s_gt_v, 1)
            vector.wait_ge(s_s2a, 1)
            vector.wait_ge(s_bias, 16)
            nc.vector.tensor_scalar_add(
                out_sb[:, 0:256], ps0[:], bias_sb[:, 0:1]
            ).then_inc(s_out0, 1)

    nc.compile()
    _STATE["nc"] = nc
    return nc


def _swizzle(a):
    """(512, X) row-major -> (128, 4*X): partition p gets rows p, 128+p, ..."""
    x = a.shape[1]
    return np.ascontiguousarray(
        a.reshape(_KC, _P, x).transpose(1, 0, 2).reshape(_P, _KC * x)
    )


def _device_scores(hs, w, bias):
    """Compute S[b, i, j] = (hs_b @ W @ hs_b^T)[i, j] + bias on 8 cores."""
    global LAST_RESULTS
    import ml_dtypes
    from concourse.bass_utils import run_bass_kernel_spmd

    nc = _build_nc()
    bf = ml_dtypes.bfloat16

    w_p = _swizzle(w.astype(bf))
    hst = [np.ascontiguousarray(hs[b].T.astype(bf)) for b in range(_B)]
    hst_p = [_swizzle(h) for h in hst]
    in_maps = []
    for c in range(_NCORES):
        b, rc = divmod(c, _GROUPS)
        r0 = rc * _RB
        in_maps.append(
            {
                "w": w_p,
                "hst": hst_p[b],
                "hsrt": _swizzle(np.ascontiguousarray(hst[b][:, r0 : r0 + _RB])),
                "bias": bias,
            }
        )

    kwargs = dict(TRACE_KWARGS) if TRACE else {}
    res = run_bass_kernel_spmd(
        nc, in_maps, core_ids=list(range(_NCORES)), trace=TRACE, **kwargs
    )
    LAST_RESULTS = res

    s = np.empty((_B, _L, _L), np.float32)
    for c in range(_NCORES):
        b, rc = divmod(c, _GROUPS)
        s[b, rc * _RB : (rc + 1) * _RB, :] = res.results[c]["out"]
    return s


def kernel(hidden_states, W, b, attention_mask, special_tokens_mask):
    hs = np.ascontiguousarray(np.asarray(hidden_states, dtype=np.float32))
    w = np.ascontiguousarray(np.asarray(W, dtype=np.float32)[0])
    bias = np.asarray(b, dtype=np.float32).reshape(1)
    am = np.asarray(attention_mask)
    sm = np.asarray(special_tokens_mask)

    # Pair indices from the (constant) masks — mirrors the reference.
    aa_mask = (am[0] == 1) & (sm[0] == 0)
    aa_positions = np.nonzero(aa_mask)[0]
    n_aa = aa_positions.shape[0]
    if n_aa < 2:
        return np.zeros((hs.shape[0], 0), dtype=np.float32)
    tri_i, tri_j = np.triu_indices(n_aa, k=1)
    idx_i = aa_positions[tri_i]
    idx_j = aa_positions[tri_j]

    if hs.shape != (_B, _L, _H) or w.shape != (_H, _H):
        # Defensive fallback for unexpected shapes (never hit by the spec).
        g = hs @ w
        s = np.einsum("bik,bjk->bij", g, hs) + bias[0]
        return s[:, idx_i, idx_j].astype(np.float32)

    s = _device_scores(hs, w, bias)  # bias already added on device
    return s[:, idx_i, idx_j].astype(np.float32)
